# revision 10
# baseline (speedup 1.0000x reference)
"""Trainium2 Bass kernel for nn_ETypePromptModel: logits = einsum('bpd,cpd->bc').

Equivalent to X @ W.T with X=[B, L*D]=[16384, 256], W=[C, L*D]=[4096, 256].
Data-parallel over B across 8 NeuronCores; label2embed replicated.

Per-core plan (B_LOC=2048):
  - Load X (2 DMAs) and W (4 DMAs) up front, descriptor-gen split across
    both HWDGE rings (sync + scalar) so transfers start early.
  - PE-transpose X and W into K-major float32r SBUF layout (fp32 has no
    DMA-transpose path); 4 transposes batched per PSUM bank -> one
    [128,512] strided copy each.
  - W handled in 4 chunks of 1024 classes; the matmul stream is
    chunk-outer so the first output DMA fires as soon as chunk 0 is
    transposed (~15us), and later chunks' transposes interleave into the
    matmul stream.
  - 256 float32r matmuls ([128k x 128b] stationary, [128k x 512c] moving),
    K=256 accumulated over 2 PSUM passes; groups of 2 PSUM banks, 3 groups
    in flight.
  - PSUM -> SBUF copies alternate Vector/Scalar engines; 64 x 512KB HWDGE
    DMA writes of the [2048, 4096] fp32 output slice.
"""

import sys

import numpy as np

sys.path.insert(0, "/opt/trn_rl_repo")

B, C, L, D = 16384, 4096, 2, 128
N_CORES = 8
B_LOC = B // N_CORES  # 2048
P = 128
N_TILE = 512  # moving free dim per matmul
M_TILES = B_LOC // P  # 16
C_TILES = C // P  # 32
W_CHUNKS = 4
C_CHUNK = C // W_CHUNKS  # 1024 classes per chunk
N_GROUP = 2  # PSUM banks per matmul accumulation group

_CACHE = {}
PROFILE = False
TRACE_ALL_CORES = False
LAST_RESULT = None


def _build():
    import concourse.mybir as mybir
    import concourse.tile as tile
    from concourse import bacc
    from concourse.masks import make_identity

    f32 = mybir.dt.float32
    f32r = mybir.dt.float32r

    nc = bacc.Bacc(
        "TRN2",
        target_bir_lowering=False,
        debug=False,
        enable_asserts=False,
        num_devices=N_CORES,
    )

    x_dram = nc.dram_tensor("batchs", [B_LOC, L, D], f32, kind="ExternalInput").ap()
    w_dram = nc.dram_tensor("label2embed", [C, L, D], f32, kind="ExternalInput").ap()
    out_dram = nc.dram_tensor("out", [B_LOC, C], f32, kind="ExternalOutput").ap()

    with tile.TileContext(nc) as tc:
        with (
            tc.tile_pool(name="const", bufs=1) as const_pool,
            tc.tile_pool(name="big", bufs=1) as big_pool,
            tc.tile_pool(name="osb", bufs=8) as out_pool,
            tc.tile_pool(name="pst", bufs=2, space="PSUM") as psum_t,
            tc.tile_pool(name="psm", bufs=6, space="PSUM") as psum_mm,
        ):
            ident = const_pool.tile([P, P], f32, name="ident")
            make_identity(nc, ident)

            _cp = [0]

            def copy(out_ap, in_ap):
                if _cp[0] % 2 == 0:
                    nc.vector.tensor_copy(out=out_ap, in_=in_ap)
                else:
                    nc.scalar.copy(out_ap, in_ap)
                _cp[0] += 1

            # ---- bulk input loads ----
            # X first on both HWDGE rings (4 chunks of 4 m-tiles), then W
            # chunks 0/1; W chunks 2/3 are triggered mid-stream so early DMA
            # bandwidth goes to the data the pipeline start needs.
            XQ = 4  # m-tiles per X chunk
            CO = C_TILES // W_CHUNKS  # 8 c-tiles per chunk
            x_stages = [
                big_pool.tile([P, XQ, L, D], f32, name=f"x_stage{xi}")
                for xi in range(M_TILES // XQ)
            ]
            w_engs = (nc.sync, nc.scalar, nc.sync, nc.scalar)
            w_stages = [
                big_pool.tile([P, CO, L, D], f32, name=f"w_stage{ci}")
                for ci in range(W_CHUNKS)
            ]

            def load_x_chunk(xi, eng):
                eng.dma_start(
                    x_stages[xi],
                    x_dram[xi * XQ * P : (xi + 1) * XQ * P].rearrange(
                        "(mo bi) p d -> bi mo p d", bi=P
                    ),
                )

            def load_w_chunk(ci):
                w_engs[ci].dma_start(
                    w_stages[ci],
                    w_dram[ci * CO * P : (ci + 1) * CO * P].rearrange(
                        "(co bi) p d -> bi co p d", bi=P
                    ),
                )

            # sync ring: X0 then W0 (the two tiles the pipeline start needs);
            # scalar ring: the rest of X, then W1. W2/W3 load mid-stream.
            load_x_chunk(0, nc.sync)
            load_x_chunk(1, nc.scalar)
            load_w_chunk(0)
            load_x_chunk(2, nc.scalar)
            load_x_chunk(3, nc.scalar)
            load_w_chunk(1)

            # ---- transposes ----
            # 4 [128,128] PE transposes batched into one PSUM bank, then one
            # [128, 2, 2, 128] strided copy out (cast to f32r).
            def transpose_batch(dst, dst_off, src, src_off, tag):
                ps = psum_t.tile([P, 2, L, P], f32, tag="tps", name=tag)
                for m1 in range(2):
                    for p in range(L):
                        nc.tensor.transpose(
                            ps[:, m1, p, :], src[:, src_off + m1, p, :], ident
                        )
                copy(
                    dst[:, :, dst_off : dst_off + 2 * P].rearrange(
                        "d p (m b) -> d p m b", m=2
                    ),
                    ps.rearrange("d m p b -> d p m b"),
                )

            # X.T per chunk: xt_chunks[q][d, p, b'] = X[q*512 + b', p, d]
            xt_chunks = [
                big_pool.tile([P, L, XQ * P], f32r, name=f"xt{xi}")
                for xi in range(M_TILES // XQ)
            ]
            for mo2 in range(M_TILES // 2):
                xi = mo2 * 2 // XQ
                transpose_batch(
                    xt_chunks[xi],
                    ((mo2 * 2) % XQ) * P,
                    x_stages[xi],
                    (mo2 * 2) % XQ,
                    "tps_x",
                )

            # W.T per chunk: wt_chunks[ci][d, p, c'] = W[ci*1024 + c', p, d]
            wt_chunks = [
                big_pool.tile([P, L, C_CHUNK], f32r, name=f"wt{ci}")
                for ci in range(W_CHUNKS)
            ]

            def w_transpose_batch(ci, co2):
                transpose_batch(
                    wt_chunks[ci], co2 * 2 * P, w_stages[ci], co2 * 2, "tps_w"
                )

            # chunk 0 fully transposed up front; chunks 1..3 interleave below
            for co2 in range(CO // 2):
                w_transpose_batch(0, co2)

            # ---- main matmul stream: chunk-outer ----
            for ci in range(W_CHUNKS):
                wt = wt_chunks[ci]
                for mt in range(M_TILES):
                    # trigger deferred W loads once the pipe is rolling
                    if ci == 0 and mt == 2:
                        load_w_chunk(2)
                    if ci == 0 and mt == 6:
                        load_w_chunk(3)
                    # spread next chunk's transposes through this phase
                    if ci < W_CHUNKS - 1 and mt % 4 == 0 and not (ci == 0 and mt < 4):
                        w_transpose_batch(ci + 1, mt // 4)
                    if ci == 0 and mt == 4:
                        w_transpose_batch(1, 0)

                    out_sb = out_pool.tile([P, C_CHUNK], f32, tag="osb", name="out_sb")
                    pms = [
                        psum_mm.tile([P, N_TILE], f32, tag="pmm", name="pmm")
                        for _ in range(N_GROUP)
                    ]
                    for p in range(L):
                        for j in range(N_GROUP):
                            nc.tensor.matmul(
                                pms[j],
                                xt_chunks[mt // XQ][:, p, (mt % XQ) * P : (mt % XQ + 1) * P],
                                wt[:, p, j * N_TILE : (j + 1) * N_TILE],
                                start=(p == 0),
                                stop=(p == L - 1),
                            )
                    for j in range(N_GROUP):
                        copy(out_sb[:, j * N_TILE : (j + 1) * N_TILE], pms[j])
                    out_eng = nc.sync if (ci * M_TILES + mt) % 2 == 0 else nc.scalar
                    out_eng.dma_start(
                        out_dram[
                            mt * P : (mt + 1) * P,
                            ci * C_CHUNK : (ci + 1) * C_CHUNK,
                        ],
                        out_sb,
                    )

    nc.compile()
    return nc


def kernel(batchs, label2embed):
    global LAST_RESULT
    from concourse.bass_utils import run_bass_kernel_spmd

    if "nc" not in _CACHE:
        _CACHE["nc"] = _build()
    nc = _CACHE["nc"]

    batchs = np.ascontiguousarray(batchs, dtype=np.float32)
    label2embed = np.ascontiguousarray(label2embed, dtype=np.float32)
    assert batchs.shape == (B, L, D) and label2embed.shape == (C, L, D)

    in_maps = [
        {
            "batchs": batchs[c * B_LOC : (c + 1) * B_LOC],
            "label2embed": label2embed,
        }
        for c in range(N_CORES)
    ]
    res = run_bass_kernel_spmd(
        nc,
        in_maps,
        core_ids=list(range(N_CORES)),
        trace=PROFILE,
        trace_cores=list(range(N_CORES)) if (PROFILE and TRACE_ALL_CORES) else None,
    )
    LAST_RESULT = res
    return np.concatenate([r["out"] for r in res.results], axis=0)
